# revision 13
# baseline (speedup 1.0000x reference)
"""DeformableConv2D (DCNv2) forward on 8 Trainium2 NeuronCores.

Data-parallel over batch: one sample per core. Per core, software-pipelined
over 8 super-groups (sg = 4 pixel blocks = 512 pixels):
  head(sg):  offset conv (PE, fp16) -> stage-B coords/weights (DVE/ACT)
             -> wrapped gather indices via PE transposes -> SWDGE desc-gen
             -> dma_gather of 2x2-patch rows
  tail(sg):  corner-weight multiply (DVE 2x-packed + gpsimd) -> accumulating
             PE transposes into banked PSUM -> im2col GEMM (PE) -> f16 store
"""
import sys
sys.path.insert(0, "/opt/trn_rl_repo")

import numpy as np
import ml_dtypes

import concourse.bass as bass
import concourse.bacc as bacc
import concourse.mybir as mybir
import concourse.tile as tile
from concourse import library_config

F32 = mybir.dt.float32
F16 = mybir.dt.float16
I16 = mybir.dt.int16
AL = mybir.AluOpType

H = W = 64
C = 128
F = 256
K = 9
PADR = 8                 # padded-coordinate margin
HP = WP = 80             # padded image
NPIX = H * W             # 4096
NBLK = 32                # pixel blocks of 128 (2 rows each)
CONVW = 66               # conv grid width (pad 1)
XCLM = 67 + 4608 + 67    # xcl with shift margins
NROWS = 2 * HP * 40      # pair-table rows = 6400
NSG = 8                  # super-groups (4 blocks each)

DY = np.repeat(np.arange(3) - 1, 3).astype(np.float32)   # per-tap dy
DX = np.tile(np.arange(3) - 1, 3).astype(np.float32)     # per-tap dx

DVE_SLOTS = 30           # corner-multiply split: slots on DVE (packed 2x)


def bcast(ap, shape):
    return ap.to_broadcast(list(shape))


_NC = None


def build_nc():
    nc = bacc.Bacc("TRN2", target_bir_lowering=False)
    xcl = nc.dram_tensor("xcl", [C, XCLM], F16, kind="ExternalInput")
    pairs = nc.dram_tensor("pairs", [NROWS, 512], F16, kind="ExternalInput")
    offk = nc.dram_tensor("offk", [C, K * 27], F16, kind="ExternalInput")
    offb = nc.dram_tensor("offb", [27, 1], F32, kind="ExternalInput")
    filt = nc.dram_tensor("filt", [C, K * 2 * 128], F16, kind="ExternalInput")
    eye32 = nc.dram_tensor("eye32", [128, 128], F32, kind="ExternalInput")
    eye16 = nc.dram_tensor("eye16", [128, 128], F16, kind="ExternalInput")
    # consts: ybase8 [128,32*9] (y+dy+8), xbase8 [128,9] (x+dx+8)
    consts = nc.dram_tensor("consts", [128, 297], F32, kind="ExternalInput")
    out_d = nc.dram_tensor("out", [2, 128, NPIX], F16, kind="ExternalOutput")

    with tile.TileContext(nc) as tc:
        with (
            tc.tile_pool(name="const", bufs=1) as cpool,
            tc.tile_pool(name="hwork", bufs=3) as hpool,       # head tiles
            tc.tile_pool(name="idxp", bufs=3) as ipool,
            tc.tile_pool(name="wpool", bufs=8) as wpool,        # idx tiles
            tc.tile_pool(name="sgpool", bufs=3) as sgpool,     # gather dst
            tc.tile_pool(name="blkpool", bufs=2) as blkpool,   # gw / cols / osb
            tc.tile_pool(name="psh", bufs=2, space="PSUM") as pshpool,
            tc.tile_pool(name="pss", bufs=2, space="PSUM") as psspool,
            tc.tile_pool(name="psc", bufs=1, space="PSUM") as pscpool,
            tc.tile_pool(name="ps2", bufs=1, space="PSUM") as ps2pool,
        ):
            nc.gpsimd.load_library(library_config.mlp)

            s_offk = cpool.tile([C, K * 27], F16)
            nc.sync.dma_start(out=s_offk[:], in_=offk[:])
            xl_tiles = []
            xl_ranges = []
            for sg in range(NSG):
                lo = (8 * sg + 1) * CONVW if sg else 0
                hi = min((8 * sg + 7) * CONVW + 266, XCLM)
                t = cpool.tile([C, hi - lo], F16, tag=f"xcl{sg}")
                nc.sync.dma_start(out=t[:], in_=xcl[:, lo:hi])
                xl_tiles.append(t)
                xl_ranges.append((lo, hi))
            s_offb = cpool.tile([27, 1], F32)
            nc.sync.dma_start(out=s_offb[:], in_=offb[:])
            s_eye32 = cpool.tile([128, 128], F32)
            nc.sync.dma_start(out=s_eye32[:], in_=eye32[:])
            s_const = cpool.tile([128, 297], F32)
            nc.sync.dma_start(out=s_const[:], in_=consts[:])
            ybase8 = s_const[:, 0:288].rearrange("p (b k) -> p b k", k=9)
            xbase8 = s_const[:, 288:297]      # [128, 9]
            s_eye16 = cpool.tile([128, 128], F16)
            nc.sync.dma_start(out=s_eye16[:], in_=eye16[:])
            s_filt = cpool.tile([C, K * 2 * 128], F16)
            nc.sync.dma_start(out=s_filt[:], in_=filt[:])

            state = {}
            idx_state = {}
            w_state = {}

            def head(sg):
                # ---- offset conv for 4 blocks + transpose to pixel-major ----
                wi_c = hpool.tile([27, 4, 128], F32, tag="wi_c")
                wiT = hpool.tile([128, 4, 27], F32, tag="wiT")
                xt = xl_tiles[sg]
                xlo = xl_ranges[sg][0]
                for bi in range(4):
                    b = 4 * sg + bi
                    q0 = (2 * b + 1) * CONVW
                    ps = pshpool.tile([27, 132], F32, tag="convps")
                    for t in range(K):
                        d = int(DY[t]) * CONVW + int(DX[t])
                        c0 = 67 + q0 + d - xlo
                        nc.tensor.matmul(
                            out=ps[:],
                            lhsT=s_offk[:, t * 27:(t + 1) * 27],
                            rhs=xt[:, c0:c0 + 132],
                            start=(t == 0), stop=(t == K - 1),
                        )
                    nc.vector.tensor_scalar(
                        out=wi_c[:, bi, :].rearrange("p (r x) -> p r x", x=64),
                        in0=ps[:].rearrange("p (r x) -> p r x", x=CONVW)[:, :, 1:65],
                        scalar1=s_offb[:, 0:1], scalar2=None, op0=AL.add)
                    pt = psspool.tile([128, 128], F32, tag="psmall")
                    nc.tensor.transpose(
                        out=pt[:, 0:27], in_=wi_c[:, bi, :],
                        identity=s_eye32[:27, :27])
                    nc.scalar.copy(out=wiT[:, bi, :], in_=pt[:, 0:27])

                # ---- stage B: coords, bilinear weights, gather indices ----
                o1 = wiT[:, :, 0:9]
                o2 = wiT[:, :, 9:18]
                mm = wiT[:, :, 18:27]
                S = [128, 4, 9]

                sigm = hpool.tile(S, F32, tag="sigm")
                nc.scalar.activation(sigm[:], mm,
                                     mybir.ActivationFunctionType.Sigmoid)

                py = hpool.tile(S, F32, tag="py")
                nc.vector.tensor_tensor(
                    out=py[:], in0=o1,
                    in1=ybase8[:, 4 * sg:4 * sg + 4, :], op=AL.add)
                nc.vector.tensor_scalar(out=py[:], in0=py[:], scalar1=2.0,
                                        scalar2=77.0, op0=AL.max, op1=AL.min)
                y0p = hpool.tile(S, F32, tag="y0p")
                nc.vector.tensor_scalar(out=y0p[:], in0=py[:],
                                        scalar1=8388607.5,
                                        scalar2=-8388608.0,
                                        op0=AL.add, op1=AL.add)

                px = hpool.tile(S, F32, tag="px")
                nc.vector.tensor_tensor(
                    out=px[:], in0=o2,
                    in1=bcast(xbase8.rearrange("p (o k) -> p o k", o=1), S),
                    op=AL.add)
                nc.vector.tensor_scalar(out=px[:], in0=px[:], scalar1=2.0,
                                        scalar2=77.0, op0=AL.max, op1=AL.min)
                x0p = hpool.tile(S, F32, tag="x0p")
                nc.vector.tensor_scalar(out=x0p[:], in0=px[:],
                                        scalar1=8388607.5,
                                        scalar2=-8388608.0,
                                        op0=AL.add, op1=AL.add)

                qx = hpool.tile(S, F32, tag="qx")
                nc.vector.tensor_scalar(out=qx[:], in0=x0p[:], scalar1=0.5,
                                        scalar2=-0.25, op0=AL.mult, op1=AL.add)
                nc.vector.tensor_scalar(out=qx[:], in0=qx[:], scalar1=8388608.0,
                                        scalar2=-8388608.0, op0=AL.add, op1=AL.add)
                parx = hpool.tile(S, F32, tag="parx")
                nc.vector.scalar_tensor_tensor(
                    out=parx[:], in0=qx[:], scalar=-2.0, in1=x0p[:],
                    op0=AL.mult, op1=AL.add)
                qy = hpool.tile(S, F32, tag="qy")
                nc.vector.tensor_scalar(out=qy[:], in0=y0p[:], scalar1=0.5,
                                        scalar2=-0.25, op0=AL.mult, op1=AL.add)
                nc.vector.tensor_scalar(out=qy[:], in0=qy[:], scalar1=8388608.0,
                                        scalar2=-8388608.0, op0=AL.add, op1=AL.add)
                pary = hpool.tile(S, F32, tag="pary")
                nc.vector.scalar_tensor_tensor(
                    out=pary[:], in0=qy[:], scalar=-2.0, in1=y0p[:],
                    op0=AL.mult, op1=AL.add)
                base = hpool.tile(S, F32, tag="base")
                nc.vector.scalar_tensor_tensor(
                    out=base[:], in0=qy[:], scalar=40.0, in1=qx[:],
                    op0=AL.mult, op1=AL.add)
                nc.vector.scalar_tensor_tensor(
                    out=base[:], in0=parx[:], scalar=1600.0, in1=base[:],
                    op0=AL.mult, op1=AL.add)
                nc.vector.scalar_tensor_tensor(
                    out=base[:], in0=pary[:], scalar=3200.0, in1=base[:],
                    op0=AL.mult, op1=AL.add)

                # wrapped-index build: idxw[q+16r, (g,pg)] = base[pg*16+q, g]
                idxw = ipool.tile([128, 288], I16, tag="idxw")
                idxw3 = idxw[:].rearrange("p (g e) -> p g e", e=8)
                pTt = psspool.tile([128, 128], F32, tag="psmall")
                pT = pTt[:36, :]
                nc.tensor.transpose(
                    out=pT, in_=base[:].rearrange("p b k -> p (b k)"),
                    identity=s_eye32[:])
                tsb = hpool.tile([36, 128], F32, tag="tsb")
                nc.vector.tensor_copy(out=tsb[:], in_=pT)
                for pg in range(8):
                    pvt = psspool.tile([128, 128], F32, tag="psmall")
                    nc.tensor.transpose(
                        out=pvt[:16, 0:36], in_=tsb[:, pg * 16:(pg + 1) * 16],
                        identity=s_eye32[:36, :36])
                    nc.vector.tensor_copy(out=idxw3[0:16, :, pg],
                                          in_=pvt[:16, 0:36])
                for r in range(1, 8):
                    nc.sync.dma_start(out=idxw[16 * r:16 * (r + 1), :],
                                      in_=idxw[0:16, :])

                idx_state[sg] = idxw

                # bilinear corner weights, f16 pairs for the 2x DVE multiply
                fy = hpool.tile(S, F32, tag="fy")
                nc.vector.tensor_tensor(out=fy[:], in0=py[:], in1=y0p[:],
                                        op=AL.subtract)
                wy0 = hpool.tile(S, F32, tag="wy0")
                nc.vector.tensor_scalar(out=wy0[:], in0=fy[:], scalar1=-1.0,
                                        scalar2=1.0, op0=AL.mult, op1=AL.add)
                fx = hpool.tile(S, F32, tag="fx")
                nc.vector.tensor_tensor(out=fx[:], in0=px[:], in1=x0p[:],
                                        op=AL.subtract)
                wx0 = hpool.tile(S, F32, tag="wx0")
                nc.vector.tensor_scalar(out=wx0[:], in0=fx[:], scalar1=-1.0,
                                        scalar2=1.0, op0=AL.mult, op1=AL.add)
                a0 = hpool.tile(S, F32, tag="a0")
                nc.vector.tensor_tensor(out=a0[:], in0=wy0[:], in1=sigm[:],
                                        op=AL.mult)
                a1 = hpool.tile(S, F32, tag="a1")
                nc.vector.tensor_tensor(out=a1[:], in0=fy[:], in1=sigm[:],
                                        op=AL.mult)
                w_f32 = hpool.tile([128, 4, 9, 2, 2], F32, tag="wf32")
                nc.vector.tensor_tensor(out=w_f32[:, :, :, 0, 0], in0=a0[:],
                                        in1=wx0[:], op=AL.mult)
                nc.vector.tensor_tensor(out=w_f32[:, :, :, 0, 1], in0=a0[:],
                                        in1=fx[:], op=AL.mult)
                nc.vector.tensor_tensor(out=w_f32[:, :, :, 1, 0], in0=a1[:],
                                        in1=wx0[:], op=AL.mult)
                nc.vector.tensor_tensor(out=w_f32[:, :, :, 1, 1], in0=a1[:],
                                        in1=fx[:], op=AL.mult)
                w2 = wpool.tile([128, 4, 36, 2], F16, tag="w2")
                wsrc = w_f32[:].rearrange("p b k y u -> p b (k y u)")
                nc.vector.tensor_copy(
                    out=w2[:, :, :, 0:1],
                    in_=wsrc.rearrange("p b (g o) -> p b g o", o=1))
                nc.vector.tensor_copy(
                    out=w2[:, :, :, 1:2],
                    in_=wsrc.rearrange("p b (g o) -> p b g o", o=1))
                w_state[sg] = w2

            def gather(sg):
                idxw = idx_state.pop(sg)
                dst = sgpool.tile([128, 36, 512], F16, tag="dst")
                for lo, ns in ((0, 8), (8, 8), (16, 8), (24, 8), (32, 4)):
                    nc.gpsimd.dma_gather(
                        dst[:, lo:lo + ns, :], pairs[:],
                        idxw[:, lo * 8:(lo + ns) * 8],
                        ns * 128, ns * 128, 512)
                state[sg] = (dst, w_state.pop(sg))

            def tail(sg):
                dst, w2 = state.pop(sg)
                cols = blkpool.tile([128, K, 512], F16, tag="cols")
                for bi in range(4):
                    gw = blkpool.tile([128, 36, 128], F16, tag="gw")
                    dsrc = dst[:, 9 * bi:9 * (bi + 1), :].rearrange(
                        "p s e -> p (s e)").rearrange(
                        "p (j c) -> p j c", c=128)
                    nd = DVE_SLOTS
                    nc.vector.tensor_tensor(
                        out=gw[:, 0:nd, :].rearrange("p j (b e) -> p j b e", e=2),
                        in0=dsrc[:, 0:nd, :].rearrange("p j (b e) -> p j b e", e=2),
                        in1=bcast(
                            w2[:, bi, 0:nd, :].rearrange(
                                "p (g o) e -> p g o e", o=1),
                            [128, nd, 64, 2]),
                        op=AL.mult)
                    nc.gpsimd.tensor_tensor(
                        out=gw[:, nd:36, :], in0=dsrc[:, nd:36, :],
                        in1=bcast(w2[:, bi, nd:36, 0:1], [128, 36 - nd, 128]),
                        op=AL.mult)
                    pcA = pscpool.tile([128, 512], F32, tag="pcA")
                    pcB = pscpool.tile([128, 512], F32, tag="pcB")
                    pcC = pscpool.tile([128, 128], F32, tag="pcC")
                    for k in range(K):
                        if k < 4:
                            pc = pcA[:, k * 128:(k + 1) * 128]
                        elif k < 8:
                            pc = pcB[:, (k - 4) * 128:(k - 3) * 128]
                        else:
                            pc = pcC[:]
                        for j in range(4):
                            nc.tensor.matmul(
                                out=pc, lhsT=gw[:, 4 * k + j, :],
                                rhs=s_eye16[:], start=(j == 0), stop=(j == 3))
                    nc.scalar.copy(
                        out=cols[:, 0:4, bi * 128:(bi + 1) * 128],
                        in_=pcA[:].rearrange("p (k c) -> p k c", c=128))
                    nc.scalar.copy(
                        out=cols[:, 4:8, bi * 128:(bi + 1) * 128],
                        in_=pcB[:].rearrange("p (k c) -> p k c", c=128))
                    nc.scalar.copy(
                        out=cols[:, 8, bi * 128:(bi + 1) * 128], in_=pcC[:])
                for fc in range(2):
                    po = ps2pool.tile([128, 512], F32, tag="outps")
                    for k in range(K):
                        nc.tensor.matmul(
                            out=po[:],
                            lhsT=s_filt[:, (k * 2 + fc) * 128:
                                        (k * 2 + fc + 1) * 128],
                            rhs=cols[:, k, :],
                            start=(k == 0), stop=(k == K - 1))
                    osb = blkpool.tile([128, 512], F16, tag="osb")
                    nc.scalar.copy(out=osb[:], in_=po[:])
                    nc.sync.dma_start(
                        out=out_d[fc, :, sg * 512:(sg + 1) * 512], in_=osb[:])

            head(0)
            gather(0)
            head(1)
            gather(1)
            head(2)
            gather(2)
            for sg in range(NSG):
                if sg + 3 < NSG:
                    head(sg + 3)
                tail(sg)
                if sg + 3 < NSG:
                    gather(sg + 3)
    nc.compile()
    return nc


def host_inputs(x, offset_kernel, offset_bias, filt_w):
    """Per-sample input maps. x [8,64,64,128] f32 etc (numpy)."""
    offk = np.ascontiguousarray(
        offset_kernel.reshape(K, C, 27).transpose(1, 0, 2).reshape(C, K * 27)
    ).astype(np.float16)
    offb = offset_bias.reshape(27, 1).astype(np.float32)
    filt_re = np.ascontiguousarray(
        filt_w.reshape(K, C, 2, 128).transpose(1, 0, 2, 3).reshape(C, K * 2 * 128)
    ).astype(np.float16)
    eye32 = np.eye(128, dtype=np.float32)
    eye16 = np.eye(128).astype(np.float16)
    consts = np.zeros((128, 297), np.float32)
    p = np.arange(128)
    yoff = p // 64
    y_all = 2 * np.arange(32)[None, :] + yoff[:, None]          # [128, 32]
    consts[:, 0:288] = (y_all[:, :, None] + DY[None, None, :]
                        + 8.0).reshape(128, 288)
    consts[:, 288:297] = (p % 64)[:, None] + DX[None, :] + 8.0

    maps = []
    for b in range(x.shape[0]):
        xp = np.zeros((HP + 2, WP + 2, C), np.float32)
        xp[PADR:PADR + H, PADR:PADR + W] = x[b]
        quad = np.zeros((2, 2, 40, 40, 2, 2, C), np.float32)
        for pY in range(2):
            for pX in range(2):
                for uy in range(2):
                    for ux in range(2):
                        quad[pY, pX, :, :, uy, ux] = \
                            xp[pY + uy:pY + uy + 80:2, pX + ux:pX + ux + 80:2]
        prs = quad.reshape(NROWS, 4 * C).astype(np.float16)

        x1 = np.zeros((CONVW, CONVW, C), np.float32)
        x1[1:65, 1:65] = x[b]
        xcl = np.zeros((C, XCLM), np.float16)
        xcl[:, 67:67 + 4356] = x1.reshape(CONVW * CONVW, C).T.astype(np.float16)
        maps.append({
            "xcl": xcl, "pairs": prs, "offk": offk, "offb": offb,
            "filt": filt_re, "eye32": eye32, "eye16": eye16, "consts": consts,
        })
    return maps


def host_output(res_list):
    outs = []
    for r in res_list:
        o = r["out"].astype(np.float32).reshape(256, NPIX)
        outs.append(np.ascontiguousarray(o.T).reshape(H, W, F))
    return np.stack(outs)


def _get_nc():
    global _NC
    if _NC is None:
        _NC = build_nc()
    return _NC


def kernel(inputs, offset_kernel, offset_bias, filt):
    from concourse.bass_utils import run_bass_kernel_spmd
    x = np.asarray(inputs, dtype=np.float32)
    maps = host_inputs(x, np.asarray(offset_kernel, np.float32),
                       np.asarray(offset_bias, np.float32),
                       np.asarray(filt, np.float32))
    nc = _get_nc()
    res = run_bass_kernel_spmd(nc, maps, core_ids=list(range(8)))
    return host_output(res.results).astype(np.float32)


# revision 15
# speedup vs baseline: 1.1935x; 1.1935x over previous
"""DeformableConv2D (DCNv2) forward on 8 Trainium2 NeuronCores.

Data-parallel over batch: one sample per core. Per core, software-pipelined
over 8 super-groups (sg = 4 pixel blocks = 512 pixels):
  head(sg):  offset conv (PE, fp16) -> stage-B coords/weights (DVE/ACT)
             -> wrapped gather indices via PE transposes -> SWDGE desc-gen
             -> dma_gather of 2x2-patch rows
  tail(sg):  corner-weight multiply (DVE 2x-packed + gpsimd) -> accumulating
             PE transposes into banked PSUM -> im2col GEMM (PE) -> f16 store
"""
import sys
sys.path.insert(0, "/opt/trn_rl_repo")

import numpy as np
import ml_dtypes

import concourse.bass as bass
import concourse.bacc as bacc
import concourse.mybir as mybir
import concourse.tile as tile
from concourse import library_config

F32 = mybir.dt.float32
F16 = mybir.dt.float16
I16 = mybir.dt.int16
AL = mybir.AluOpType

H = W = 64
C = 128
F = 256
K = 9
PADR = 8                 # padded-coordinate margin
HP = WP = 80             # padded image
NPIX = H * W             # 4096
NBLK = 32                # pixel blocks of 128 (2 rows each)
CONVW = 66               # conv grid width (pad 1)
XCLM = 67 + 4608 + 67    # xcl with shift margins
NROWS = 2 * HP * 40      # pair-table rows = 6400
NSG = 8                  # super-groups (4 blocks each)

DY = np.repeat(np.arange(3) - 1, 3).astype(np.float32)   # per-tap dy
DX = np.tile(np.arange(3) - 1, 3).astype(np.float32)     # per-tap dx

DVE_SLOTS = 30           # corner-multiply split: slots on DVE (packed 2x)


def bcast(ap, shape):
    return ap.to_broadcast(list(shape))


_NC = None


def build_nc():
    nc = bacc.Bacc("TRN2", target_bir_lowering=False)
    xcl = nc.dram_tensor("xcl", [C, XCLM], F16, kind="ExternalInput")
    pairs = nc.dram_tensor("pairs", [NROWS, 512], F16, kind="ExternalInput")
    offk = nc.dram_tensor("offk", [C, K * 27], F16, kind="ExternalInput")
    offb = nc.dram_tensor("offb", [27, 1], F32, kind="ExternalInput")
    filt = nc.dram_tensor("filt", [C, K * 2 * 128], F16, kind="ExternalInput")
    eye32 = nc.dram_tensor("eye32", [128, 128], F32, kind="ExternalInput")
    eye16 = nc.dram_tensor("eye16", [128, 128], F16, kind="ExternalInput")
    # consts: ybase8 [128,32*9] (y+dy+8), xbase8 [128,9] (x+dx+8)
    consts = nc.dram_tensor("consts", [128, 297], F32, kind="ExternalInput")
    out_d = nc.dram_tensor("out", [2, 128, NPIX], F16, kind="ExternalOutput")

    with tile.TileContext(nc) as tc:
        with (
            tc.tile_pool(name="const", bufs=1) as cpool,
            tc.tile_pool(name="hwork", bufs=3) as hpool,       # head tiles
            tc.tile_pool(name="idxp", bufs=3) as ipool,
            tc.tile_pool(name="wpool", bufs=8) as wpool,        # idx tiles
            tc.tile_pool(name="sgpool", bufs=3) as sgpool,     # gather dst
            tc.tile_pool(name="blkpool", bufs=2) as blkpool,   # cols / osb
            tc.tile_pool(name="gwp", bufs=3) as gwpool,
            tc.tile_pool(name="psh", bufs=2, space="PSUM") as pshpool,
            tc.tile_pool(name="pss", bufs=2, space="PSUM") as psspool,
            tc.tile_pool(name="psc", bufs=1, space="PSUM") as pscpool,
            tc.tile_pool(name="ps2", bufs=1, space="PSUM") as ps2pool,
        ):
            nc.gpsimd.load_library(library_config.mlp)

            s_offk = cpool.tile([C, K * 27], F16)
            nc.sync.dma_start(out=s_offk[:], in_=offk[:])
            xl_tiles = []
            xl_ranges = []
            for sg in range(NSG):
                lo = (8 * sg + 1) * CONVW if sg else 0
                hi = min((8 * sg + 7) * CONVW + 266, XCLM)
                t = cpool.tile([C, hi - lo], F16, tag=f"xcl{sg}")
                nc.sync.dma_start(out=t[:], in_=xcl[:, lo:hi])
                xl_tiles.append(t)
                xl_ranges.append((lo, hi))
            s_offb = cpool.tile([27, 1], F32)
            nc.sync.dma_start(out=s_offb[:], in_=offb[:])
            s_eye32 = cpool.tile([128, 128], F32)
            nc.sync.dma_start(out=s_eye32[:], in_=eye32[:])
            s_const = cpool.tile([128, 297], F32)
            nc.sync.dma_start(out=s_const[:], in_=consts[:])
            ybase8 = s_const[:, 0:288].rearrange("p (b k) -> p b k", k=9)
            xbase8 = s_const[:, 288:297]      # [128, 9]
            s_eye16 = cpool.tile([128, 128], F16)
            nc.sync.dma_start(out=s_eye16[:], in_=eye16[:])
            s_filt = cpool.tile([C, K * 2 * 128], F16)
            nc.sync.dma_start(out=s_filt[:], in_=filt[:])

            hstate = {}
            idx_state = {}
            w_state = {}
            state = {}

            def headA(sg):
                # ---- offset conv for 4 blocks + transpose to pixel-major ----
                wi_c = hpool.tile([27, 4, 128], F32, tag="wi_c")
                wiT = hpool.tile([128, 4, 27], F32, tag="wiT")
                xt = xl_tiles[sg]
                xlo = xl_ranges[sg][0]
                for bi in range(4):
                    b = 4 * sg + bi
                    q0 = (2 * b + 1) * CONVW
                    ps = pshpool.tile([27, 132], F32, tag="convps")
                    for t in range(K):
                        d = int(DY[t]) * CONVW + int(DX[t])
                        c0 = 67 + q0 + d - xlo
                        nc.tensor.matmul(
                            out=ps[:],
                            lhsT=s_offk[:, t * 27:(t + 1) * 27],
                            rhs=xt[:, c0:c0 + 132],
                            start=(t == 0), stop=(t == K - 1),
                        )
                    nc.vector.tensor_scalar(
                        out=wi_c[:, bi, :].rearrange("p (r x) -> p r x", x=64),
                        in0=ps[:].rearrange("p (r x) -> p r x", x=CONVW)[:, :, 1:65],
                        scalar1=s_offb[:, 0:1], scalar2=None, op0=AL.add)
                    pt = psspool.tile([128, 128], F32, tag="psmall")
                    nc.tensor.transpose(
                        out=pt[:, 0:27], in_=wi_c[:, bi, :],
                        identity=s_eye32[:27, :27])
                    nc.scalar.copy(out=wiT[:, bi, :], in_=pt[:, 0:27])

                # ---- stage B critical path: coords -> pair-row index ----
                o1 = wiT[:, :, 0:9]
                o2 = wiT[:, :, 9:18]
                mm = wiT[:, :, 18:27]
                S = [128, 4, 9]

                sigm = hpool.tile(S, F32, tag="sigm")
                nc.scalar.activation(sigm[:], mm,
                                     mybir.ActivationFunctionType.Sigmoid)

                py = hpool.tile(S, F32, tag="py")
                nc.vector.tensor_tensor(
                    out=py[:], in0=o1,
                    in1=ybase8[:, 4 * sg:4 * sg + 4, :], op=AL.add)
                nc.vector.tensor_scalar(out=py[:], in0=py[:], scalar1=2.0,
                                        scalar2=77.0, op0=AL.max, op1=AL.min)
                y0p = hpool.tile(S, F32, tag="y0p")
                nc.vector.tensor_scalar(out=y0p[:], in0=py[:],
                                        scalar1=8388607.5,
                                        scalar2=-8388608.0,
                                        op0=AL.add, op1=AL.add)

                px = hpool.tile(S, F32, tag="px")
                nc.vector.tensor_tensor(
                    out=px[:], in0=o2,
                    in1=bcast(xbase8.rearrange("p (o k) -> p o k", o=1), S),
                    op=AL.add)
                nc.vector.tensor_scalar(out=px[:], in0=px[:], scalar1=2.0,
                                        scalar2=77.0, op0=AL.max, op1=AL.min)
                x0p = hpool.tile(S, F32, tag="x0p")
                nc.vector.tensor_scalar(out=x0p[:], in0=px[:],
                                        scalar1=8388607.5,
                                        scalar2=-8388608.0,
                                        op0=AL.add, op1=AL.add)

                qx = hpool.tile(S, F32, tag="qx")
                nc.vector.tensor_scalar(out=qx[:], in0=x0p[:], scalar1=0.5,
                                        scalar2=-0.25, op0=AL.mult, op1=AL.add)
                nc.vector.tensor_scalar(out=qx[:], in0=qx[:], scalar1=8388608.0,
                                        scalar2=-8388608.0, op0=AL.add, op1=AL.add)
                parx = hpool.tile(S, F32, tag="parx")
                nc.vector.scalar_tensor_tensor(
                    out=parx[:], in0=qx[:], scalar=-2.0, in1=x0p[:],
                    op0=AL.mult, op1=AL.add)
                qy = hpool.tile(S, F32, tag="qy")
                nc.vector.tensor_scalar(out=qy[:], in0=y0p[:], scalar1=0.5,
                                        scalar2=-0.25, op0=AL.mult, op1=AL.add)
                nc.vector.tensor_scalar(out=qy[:], in0=qy[:], scalar1=8388608.0,
                                        scalar2=-8388608.0, op0=AL.add, op1=AL.add)
                pary = hpool.tile(S, F32, tag="pary")
                nc.vector.scalar_tensor_tensor(
                    out=pary[:], in0=qy[:], scalar=-2.0, in1=y0p[:],
                    op0=AL.mult, op1=AL.add)
                base = hpool.tile(S, F32, tag="base")
                nc.vector.scalar_tensor_tensor(
                    out=base[:], in0=qy[:], scalar=40.0, in1=qx[:],
                    op0=AL.mult, op1=AL.add)
                nc.vector.scalar_tensor_tensor(
                    out=base[:], in0=parx[:], scalar=1600.0, in1=base[:],
                    op0=AL.mult, op1=AL.add)
                nc.vector.scalar_tensor_tensor(
                    out=base[:], in0=pary[:], scalar=3200.0, in1=base[:],
                    op0=AL.mult, op1=AL.add)
                hstate[sg] = (base, py, px, y0p, x0p, sigm)

            def idxT(sg):
                # wrapped-index build: idxw[q+16r, (g,pg)] = base[pg*16+q, g]
                base, py, px, y0p, x0p, sigm = hstate.pop(sg)
                S = [128, 4, 9]
                idxw = ipool.tile([128, 288], I16, tag="idxw")
                idxw3 = idxw[:].rearrange("p (g e) -> p g e", e=8)
                pTt = psspool.tile([128, 128], F32, tag="psmall")
                pT = pTt[:36, :]
                nc.tensor.transpose(
                    out=pT, in_=base[:].rearrange("p b k -> p (b k)"),
                    identity=s_eye32[:])
                tsb = hpool.tile([36, 128], F32, tag="tsb")
                nc.vector.tensor_copy(out=tsb[:], in_=pT)
                for pg in range(8):
                    pvt = psspool.tile([128, 128], F32, tag="psmall")
                    nc.tensor.transpose(
                        out=pvt[:16, 0:36], in_=tsb[:, pg * 16:(pg + 1) * 16],
                        identity=s_eye32[:36, :36])
                    nc.scalar.copy(out=idxw3[0:16, :, pg],
                                   in_=pvt[:16, 0:36])
                for r in range(1, 8):
                    nc.sync.dma_start(out=idxw[16 * r:16 * (r + 1), :],
                                      in_=idxw[0:16, :])
                idx_state[sg] = idxw

                # bilinear corner weights, f16 pairs for the 2x DVE multiply
                fy = hpool.tile(S, F32, tag="fy")
                nc.vector.tensor_tensor(out=fy[:], in0=py[:], in1=y0p[:],
                                        op=AL.subtract)
                wy0 = hpool.tile(S, F32, tag="wy0")
                nc.vector.tensor_scalar(out=wy0[:], in0=fy[:], scalar1=-1.0,
                                        scalar2=1.0, op0=AL.mult, op1=AL.add)
                fx = hpool.tile(S, F32, tag="fx")
                nc.vector.tensor_tensor(out=fx[:], in0=px[:], in1=x0p[:],
                                        op=AL.subtract)
                wx0 = hpool.tile(S, F32, tag="wx0")
                nc.vector.tensor_scalar(out=wx0[:], in0=fx[:], scalar1=-1.0,
                                        scalar2=1.0, op0=AL.mult, op1=AL.add)
                a0 = hpool.tile(S, F32, tag="a0")
                nc.vector.tensor_tensor(out=a0[:], in0=wy0[:], in1=sigm[:],
                                        op=AL.mult)
                a1 = hpool.tile(S, F32, tag="a1")
                nc.vector.tensor_tensor(out=a1[:], in0=fy[:], in1=sigm[:],
                                        op=AL.mult)
                w_f32 = hpool.tile([128, 4, 9, 2, 2], F32, tag="wf32")
                nc.vector.tensor_tensor(out=w_f32[:, :, :, 0, 0], in0=a0[:],
                                        in1=wx0[:], op=AL.mult)
                nc.vector.tensor_tensor(out=w_f32[:, :, :, 0, 1], in0=a0[:],
                                        in1=fx[:], op=AL.mult)
                nc.vector.tensor_tensor(out=w_f32[:, :, :, 1, 0], in0=a1[:],
                                        in1=wx0[:], op=AL.mult)
                nc.vector.tensor_tensor(out=w_f32[:, :, :, 1, 1], in0=a1[:],
                                        in1=fx[:], op=AL.mult)
                w2 = wpool.tile([128, 4, 36, 2], F16, tag="w2")
                wsrc = w_f32[:].rearrange("p b k y u -> p b (k y u)")
                nc.vector.tensor_copy(
                    out=w2[:, :, :, 0:1],
                    in_=wsrc.rearrange("p b (g o) -> p b g o", o=1))
                nc.vector.tensor_copy(
                    out=w2[:, :, :, 1:2],
                    in_=wsrc.rearrange("p b (g o) -> p b g o", o=1))
                w_state[sg] = w2

            def gather(sg):
                idxw = idx_state.pop(sg)
                dst = sgpool.tile([128, 36, 512], F16, tag="dst")
                for lo, ns in ((0, 8), (8, 8), (16, 8), (24, 8), (32, 4)):
                    nc.gpsimd.dma_gather(
                        dst[:, lo:lo + ns, :], pairs[:],
                        idxw[:, lo * 8:(lo + ns) * 8],
                        ns * 128, ns * 128, 512)
                state[sg] = (dst, w_state.pop(sg))

            def tail(sg):
                dst, w2 = state.pop(sg)
                cols = blkpool.tile([128, K, 512], F16, tag="cols")
                for bi in range(4):
                    gw = gwpool.tile([128, 36, 128], F16, tag="gw")
                    dsrc = dst[:, 9 * bi:9 * (bi + 1), :].rearrange(
                        "p s e -> p (s e)").rearrange(
                        "p (j c) -> p j c", c=128)
                    nc.vector.tensor_tensor(
                        out=gw[:].rearrange("p j (b e) -> p j b e", e=2),
                        in0=dsrc[:].rearrange("p j (b e) -> p j b e", e=2),
                        in1=bcast(
                            w2[:, bi, :, :].rearrange(
                                "p (g o) e -> p g o e", o=1),
                            [128, 36, 64, 2]),
                        op=AL.mult)
                    pcA = pscpool.tile([128, 512], F32, tag="pcA")
                    pcB = pscpool.tile([128, 512], F32, tag="pcB")
                    pcC = pscpool.tile([128, 128], F32, tag="pcC")
                    for k in range(K):
                        if k < 4:
                            pc = pcA[:, k * 128:(k + 1) * 128]
                        elif k < 8:
                            pc = pcB[:, (k - 4) * 128:(k - 3) * 128]
                        else:
                            pc = pcC[:]
                        for j in range(4):
                            nc.tensor.matmul(
                                out=pc, lhsT=gw[:, 4 * k + j, :],
                                rhs=s_eye16[:], start=(j == 0), stop=(j == 3))
                    nc.scalar.copy(
                        out=cols[:, 0:4, bi * 128:(bi + 1) * 128],
                        in_=pcA[:].rearrange("p (k c) -> p k c", c=128))
                    nc.scalar.copy(
                        out=cols[:, 4:8, bi * 128:(bi + 1) * 128],
                        in_=pcB[:].rearrange("p (k c) -> p k c", c=128))
                    nc.scalar.copy(
                        out=cols[:, 8, bi * 128:(bi + 1) * 128], in_=pcC[:])
                for fc in range(2):
                    po = ps2pool.tile([128, 512], F32, tag="outps")
                    for k in range(K):
                        nc.tensor.matmul(
                            out=po[:],
                            lhsT=s_filt[:, (k * 2 + fc) * 128:
                                        (k * 2 + fc + 1) * 128],
                            rhs=cols[:, k, :],
                            start=(k == 0), stop=(k == K - 1))
                    osb = blkpool.tile([128, 512], F16, tag="osb")
                    nc.scalar.copy(out=osb[:], in_=po[:])
                    nc.sync.dma_start(
                        out=out_d[fc, :, sg * 512:(sg + 1) * 512], in_=osb[:])

            for sg in range(3):
                headA(sg)
                idxT(sg)
                gather(sg)
            for sg in range(NSG):
                if sg + 3 < NSG:
                    headA(sg + 3)
                tail(sg)
                if sg + 3 < NSG:
                    idxT(sg + 3)
                    gather(sg + 3)
    nc.compile()
    return nc


def host_inputs(x, offset_kernel, offset_bias, filt_w):
    """Per-sample input maps. x [8,64,64,128] f32 etc (numpy)."""
    offk = np.ascontiguousarray(
        offset_kernel.reshape(K, C, 27).transpose(1, 0, 2).reshape(C, K * 27)
    ).astype(np.float16)
    offb = offset_bias.reshape(27, 1).astype(np.float32)
    filt_re = np.ascontiguousarray(
        filt_w.reshape(K, C, 2, 128).transpose(1, 0, 2, 3).reshape(C, K * 2 * 128)
    ).astype(np.float16)
    eye32 = np.eye(128, dtype=np.float32)
    eye16 = np.eye(128).astype(np.float16)
    consts = np.zeros((128, 297), np.float32)
    p = np.arange(128)
    yoff = p // 64
    y_all = 2 * np.arange(32)[None, :] + yoff[:, None]          # [128, 32]
    consts[:, 0:288] = (y_all[:, :, None] + DY[None, None, :]
                        + 8.0).reshape(128, 288)
    consts[:, 288:297] = (p % 64)[:, None] + DX[None, :] + 8.0

    maps = []
    for b in range(x.shape[0]):
        xp = np.zeros((HP + 2, WP + 2, C), np.float32)
        xp[PADR:PADR + H, PADR:PADR + W] = x[b]
        quad = np.zeros((2, 2, 40, 40, 2, 2, C), np.float32)
        for pY in range(2):
            for pX in range(2):
                for uy in range(2):
                    for ux in range(2):
                        quad[pY, pX, :, :, uy, ux] = \
                            xp[pY + uy:pY + uy + 80:2, pX + ux:pX + ux + 80:2]
        prs = quad.reshape(NROWS, 4 * C).astype(np.float16)

        x1 = np.zeros((CONVW, CONVW, C), np.float32)
        x1[1:65, 1:65] = x[b]
        xcl = np.zeros((C, XCLM), np.float16)
        xcl[:, 67:67 + 4356] = x1.reshape(CONVW * CONVW, C).T.astype(np.float16)
        maps.append({
            "xcl": xcl, "pairs": prs, "offk": offk, "offb": offb,
            "filt": filt_re, "eye32": eye32, "eye16": eye16, "consts": consts,
        })
    return maps


def host_output(res_list):
    outs = []
    for r in res_list:
        o = r["out"].astype(np.float32).reshape(256, NPIX)
        outs.append(np.ascontiguousarray(o.T).reshape(H, W, F))
    return np.stack(outs)


def _get_nc():
    global _NC
    if _NC is None:
        _NC = build_nc()
    return _NC


def kernel(inputs, offset_kernel, offset_bias, filt):
    from concourse.bass_utils import run_bass_kernel_spmd
    x = np.asarray(inputs, dtype=np.float32)
    maps = host_inputs(x, np.asarray(offset_kernel, np.float32),
                       np.asarray(offset_bias, np.float32),
                       np.asarray(filt, np.float32))
    nc = _get_nc()
    res = run_bass_kernel_spmd(nc, maps, core_ids=list(range(8)))
    return host_output(res.results).astype(np.float32)
